# revision 5
# baseline (speedup 1.0000x reference)
"""Trainium2 Bass kernel v2 for nn_CELossWithSVLS_VE (SVLS cross-entropy).

Math (as v1): per ±offset-pair trick with
  u_n = exp(-0.5*(maxdiff_n^2 + r_n^2)),  w_center = 1/2, w_n = u_n/(2S),
  loss(v) = lse(v) - 0.5*y(v) - 0.5*rS(v)*T(v).

v2 engine plan (cost-model driven; neuronxcc-validated op support):
- Host precomputes h-shifted images AND one-hot masks (4th channel == 1),
  all bf16: no on-device is_equal, no SBUF-shift DMAs, no Sh matrices.
- PHASE-MAJOR schedule: all channel-difference subs are emitted first
  into persistent tiles (PE via eye/negI matmul pairs + Act copybacks;
  Pool via gpsimd TT-subtract, its only fast ALU ops being add/sub/mult),
  so no engine head-of-line blocks another mid-pipeline.
- DVE (critical engine; all ops bf16 packed SBUF = 2x mode) keeps the
  max/min trees, mask products, and final combine; the logits phase is
  emitted first so DVE/Act start before the subs land.
- Act: squares+exps batched per equal-r2 pair group, lse, copybacks,
  PSUM readout.
- Final: accP[4] = [acc1..3, S]; dx4 = [dx1..3, x0]; T = sum(acc4*dx4);
  per-partition partials via accum_out (lse / -T/(2S) / y) -> host sum.
"""
import os
import sys
from contextlib import ExitStack

import numpy as np

if "/opt/trn_rl_repo" not in sys.path:
    sys.path.insert(0, "/opt/trn_rl_repo")

B, C, D, H, W = 2, 4, 64, 64, 64
NCORES = 8
DL = D // NCORES          # 8 local d-planes
DE, WE = DL + 2, W + 2    # 10, 66 (d/w halos)
P = 128                   # partitions = (b, h)
NVOX = B * D * H * W      # 524288

# 13 positive offsets; r2 = i*i+j*j+k*k sets the exp bias.
# Indices 0-3: j==0 (extended-box shared sub); 4-12: j!=0.
PAIRS = [
    (1, 0, 0), (0, 0, 1), (1, 0, 1), (1, 0, -1),
    (0, 1, 0), (1, 1, 0), (1, -1, 0), (0, 1, 1), (0, 1, -1),
    (1, 1, 1), (1, 1, -1), (1, -1, 1), (1, -1, -1),
]

# Engine assignment (pair indices).
def _envset(name, default):
    raw = os.environ.get(name)
    if raw is None:
        return frozenset(default)
    return frozenset(int(v) for v in raw.split(",") if v != "")


PE_MAX = frozenset(range(4, 13))      # j!=0 max-frame subs on PE
PE_MIN = _envset("V2_PE_MIN", {6})               # min-frame subs on PE
POOL_J0 = _envset("V2_POOL_J0", {0, 1})          # j==0 ext-box subs on Pool
POOL_MIN = frozenset(range(4, 13)) - PE_MIN      # rest of min-subs on Pool
# Consumer order: PE-sub pairs first (ready earliest), then Pool pairs in
# Pool program order.  Batches group consecutive equal-r2 pairs for shared
# square+exp activations.
_BATCH_DEF = "4|5,6|2,3|0,1|7,8|9,10|11,12"
BATCHES = [[int(v) for v in grp.split(",")]
           for grp in os.environ.get("V2_BATCHES", _BATCH_DEF).split("|")]

LSE_ACCUM = os.environ.get("V2_LSE_ACCUM", "1") == "1"
USE_TTR = os.environ.get("V2_TTR", "0") == "1"
RECIP_PSUM = os.environ.get("V2_RECIP_PSUM", "0") == "1"
TAB6 = os.environ.get("V2_TAB6", "1") == "1"

_CACHED = {}


def _build_nc():
    import concourse.bacc as bacc
    import concourse.mybir as mybir
    import concourse.tile as tile

    AF = mybir.ActivationFunctionType
    ALU = mybir.AluOpType
    dt = mybir.dt

    nc = bacc.Bacc("TRN2", target_bir_lowering=False, debug=False,
                   num_devices=NCORES)
    imgc_d = nc.dram_tensor("imgc", [P, C * DE * WE], dt.bfloat16,
                            kind="ExternalInput")
    imghp_d = nc.dram_tensor("imghp", [P, C * DE * WE], dt.bfloat16,
                             kind="ExternalInput")
    imghm_d = nc.dram_tensor("imghm", [P, C * DE * WE], dt.bfloat16,
                             kind="ExternalInput")
    mskc_d = nc.dram_tensor("mskc", [P, 4 * DE * WE], dt.bfloat16,
                            kind="ExternalInput")
    mskhp_d = nc.dram_tensor("mskhp", [P, 3 * DE * WE], dt.bfloat16,
                             kind="ExternalInput")
    mskhm_d = nc.dram_tensor("mskhm", [P, 3 * DE * WE], dt.bfloat16,
                             kind="ExternalInput")
    logit_d = nc.dram_tensor("logits", [P, C * DL * W], dt.bfloat16,
                             kind="ExternalInput")
    eye_d = nc.dram_tensor("eye", [P, 2 * P], dt.bfloat16,
                           kind="ExternalInput")
    out_d = nc.dram_tensor("out", [P, 3], dt.float32, kind="ExternalOutput")

    import concourse.bass as bass_mod

    with tile.TileContext(nc) as tc, ExitStack() as ctx:
        persist = ctx.enter_context(tc.tile_pool(name="persist", bufs=1))
        subsp = ctx.enter_context(tc.tile_pool(name="subsp", bufs=1))
        cpool = ctx.enter_context(tc.tile_pool(name="cpool", bufs=1))
        trans = ctx.enter_context(tc.tile_pool(name="trans", bufs=int(os.environ.get("V2_TRANS", "2"))))
        upool = ctx.enter_context(tc.tile_pool(name="upool", bufs=int(os.environ.get("V2_UPOOL", "2"))))
        psum = ctx.enter_context(
            tc.tile_pool(name="psum", bufs=1, space=bass_mod.MemorySpace.PSUM))
        psum2 = ctx.enter_context(
            tc.tile_pool(name="psum2", bufs=2, space=bass_mod.MemorySpace.PSUM))
        dhpool = ctx.enter_context(
            tc.tile_pool(name="dhpool",
                         bufs=int(os.environ.get("V2_DHBUFS", "3"))))

        f32, bf16 = dt.float32, dt.bfloat16
        TT = nc.vector.tensor_tensor

        if TAB6:
            nc.scalar.add_instruction(mybir.InstLoadActFuncSet(
                name=nc.get_next_instruction_name(), act_func_set_id=6,
                ins=[], outs=[]))

        # ---- loads.  SP queue carries the critical-path-early tensors in
        # priority order; DVE/Act queues carry the h-shifted copies so the
        # three HWDGE queues fill SBUF in parallel. ----
        EARLY = os.environ.get("V2_EARLY", "0") == "1"
        IMGFIRST = os.environ.get("V2_IMGFIRST", "0") == "1"
        imgb = persist.tile([P, C, DE, WE], bf16, tag="imgb")
        if IMGFIRST:
            nc.sync.dma_start(imgb[:], imgc_d[:, :])
        x = persist.tile([P, C, DL, W], bf16, tag="x")
        if EARLY:
            nc.sync.dma_start(x[:, 0:2], logit_d[:, 0:2 * DL * W])
            nc.sync.dma_start(x[:, 2:4], logit_d[:, 2 * DL * W:4 * DL * W])
        else:
            nc.sync.dma_start(x[:], logit_d[:, :])
        if not IMGFIRST:
            nc.sync.dma_start(imgb[:], imgc_d[:, :])
        mats = persist.tile([P, 2, P], bf16, tag="mats")
        nc.sync.dma_start(mats[:], eye_d[:, :])
        dx4 = persist.tile([P, 4, DL, W], bf16, tag="dx4")
        nc.sync.dma_start(dx4[:, 3], logit_d[:, 0:DL * W])  # x0 channel
        imgb_hp = persist.tile([P, C, DE, WE], bf16, tag="imgb_hp")
        nc.scalar.dma_start(imgb_hp[:], imghp_d[:, :])
        imgb_hm = persist.tile([P, C, DE, WE], bf16, tag="imgb_hm")
        nc.scalar.dma_start(imgb_hm[:], imghm_d[:, :])
        masks = persist.tile([P, 4, DE, WE], bf16, tag="masks")
        (nc.scalar if EARLY else nc.sync).dma_start(masks[:], mskc_d[:, :])
        masks_hp = persist.tile([P, 3, DE, WE], bf16, tag="masks_hp")
        nc.scalar.dma_start(masks_hp[:], mskhp_d[:, :])
        masks_hm = persist.tile([P, 3, DE, WE], bf16, tag="masks_hm")
        nc.scalar.dma_start(masks_hm[:], mskhm_d[:, :])

        img_h = {1: imgb_hp, 0: imgb, -1: imgb_hm}
        msk_h = {1: masks_hp, 0: masks, -1: masks_hm}
        eye, negI = mats[:, 0], mats[:, 1]

        def cv(tile_, i, k, ch=None):
            """center view shifted by (i, ., k) of a [..., DE, WE] tile."""
            if ch is None:
                return tile_[:, :, 1 + i:1 + i + DL, 1 + k:1 + k + W]
            return tile_[:, ch, 1 + i:1 + i + DL, 1 + k:1 + k + W]

        bias_t = {}
        for r2 in (1.0, 2.0, 3.0):
            bt = persist.tile([P, 1], f32, tag=f"bias{int(r2)}")
            nc.gpsimd.memset(bt[:], -0.5 * r2)
            bias_t[r2] = bt

        # ---- warm-up phase: logits-only work (x DMA lands first) ----
        WARMUP_MARK = True
        part = cpool.tile([P, 3], f32, tag="part")
        expx = cpool.tile([P, C, DL, W], bf16, tag="expx")
        if EARLY:
            nc.scalar.activation(expx[:, 0:2], x[:, 0:2], AF.Exp)
            nc.scalar.activation(expx[:, 2:4], x[:, 2:4], AF.Exp)
        else:
            nc.scalar.activation(expx[:], x[:], AF.Exp)
        TT(dx4[:, 0:3], x[:, 1:4],
           x[:, 0:1].broadcast_to((P, 3, DL, W)), ALU.subtract)

        # ---- sub phase: all channel-difference arrays, PE + Pool ----
        subt = {}  # (pi, fr) -> ('halves', [hhA, hhB]) | ('view', fn)

        def pe_sub(pi, fr, pos_view, neg_view):
            d4h = []
            for half in range(2):
                d4p = psum2.tile([P, 2, DL, W], f32, tag="d4p")
                for cc in range(2):
                    c = 2 * half + cc
                    nc.tensor.matmul(d4p[:, cc], eye, pos_view(c),
                                     start=True, stop=False)
                    nc.tensor.matmul(d4p[:, cc], negI, neg_view(c),
                                     start=False, stop=True)
                if os.environ.get("V2_D4HRING", "0") == "1":
                    hh = dhpool.tile([P, 2, DL, W], bf16,
                                     tag=f"d4h{half}")
                else:
                    hh = subsp.tile([P, 2, DL, W], bf16,
                                    tag=f"d4h_{pi}_{fr}_{half}")
                nc.scalar.copy(hh[:], d4p[:])
                d4h.append(hh)
            subt[(pi, fr)] = ("halves", d4h)

        def pe_subs_for(batch):
            for pi in batch:
                i, j, k = PAIRS[pi]
                if pi in PE_MAX:
                    pe_sub(pi, 0,
                           lambda c, j=j, i=i, k=k: cv(img_h[j], i, k, c),
                           lambda c: cv(imgb, 0, 0, c))
                if pi in PE_MIN:
                    pe_sub(pi, 1, lambda c: cv(imgb, 0, 0, c),
                           lambda c, j=j, i=i, k=k: cv(img_h[-j], -i, -k, c))

        # Subs on Pool get a channel split (Pool writes [0:SPLIT), DVE the
        # rest into the same tile) so Pool's serial per-sub latency shrinks;
        # the tree's first level consumes both halves naturally.
        SPLIT = int(os.environ.get("V2_SPLIT", "4"))

        def split_sub(dst, pos, neg, on_pool):
            if on_pool and SPLIT < C:
                nc.gpsimd.tensor_tensor(dst[:, 0:SPLIT], pos[:, 0:SPLIT],
                                        neg[:, 0:SPLIT], ALU.subtract)
                TT(dst[:, SPLIT:C], pos[:, SPLIT:C], neg[:, SPLIT:C],
                   ALU.subtract)
            else:
                eng = nc.gpsimd if on_pool else nc.vector
                eng.tensor_tensor(dst[:], pos[:], neg[:], ALU.subtract)

        PESUBS_EARLY = os.environ.get("V2_PESUBS_EARLY", "0") == "1"
        if PESUBS_EARLY:
            pe_need = sorted(set(PE_MAX) | set(PE_MIN))
            pe_subs_for(pe_need)

        pool_order = [pi for b in BATCHES for pi in b
                      if pi in POOL_J0 or pi in POOL_MIN or
                      (pi < 4 and pi not in POOL_J0)]
        for pi in pool_order:
            i, j, k = PAIRS[pi]
            if pi < 4:
                nd, nw = (9 if i else 8), (65 if k else 64)
                d0, w0 = (0 if i == 1 else 1), (0 if k == 1 else 1)
                dpe = subsp.tile([P, C, nd, nw], bf16, tag=f"dpe_{pi}")
                split_sub(dpe,
                          imgb[:, :, d0 + i:d0 + i + nd, w0 + k:w0 + k + nw],
                          imgb[:, :, d0:d0 + nd, w0:w0 + nw], pi in POOL_J0)

                def j0_view(fr, dpe=dpe, d0=d0, w0=w0, i=i, k=k):
                    ds = 1 - d0 - (i if fr else 0)
                    ws = 1 - w0 - (k if fr else 0)
                    return dpe[:, :, ds:ds + DL, ws:ws + W]
                subt[(pi, 0)] = ("view", j0_view(0))
                subt[(pi, 1)] = ("view", j0_view(1))
            else:
                d4m = subsp.tile([P, C, DL, W], bf16, tag=f"d4m_{pi}")
                split_sub(d4m, cv(imgb, 0, 0), cv(img_h[-j], -i, -k), True)
                subt[(pi, 1)] = ("view", d4m[:])

        # ---- consumer loop: trees -> batched sq+exp -> prods+accum ----
        accP = psum.tile([P, 4, DL, W], f32, tag="accP")
        NFR = 2 * len(PAIRS)

        def tree(dst, hi, lo, op):
            m2 = trans.tile([P, 2, DL, W], bf16, tag="m2")
            TT(m2[:], hi, lo, op)
            TT(dst, m2[:, 0], m2[:, 1], op)

        tcount = 0

        def trees_sq_exp(batch):
            nb = len(batch)
            r2 = float(sum(v * v for v in PAIRS[batch[0]]))
            m1p = trans.tile([P, 2 * nb, DL, W], bf16, tag=f"m1p{nb}")
            for bi, pi in enumerate(batch):
                for fr in range(2):
                    mop = ALU.max if fr == 0 else ALU.min
                    kind, val = subt[(pi, fr)]
                    dst = m1p[:, 2 * bi + fr]
                    if kind == "halves":
                        tree(dst, val[0][:], val[1][:], mop)
                    else:
                        tree(dst, val[:, 0:2], val[:, 2:4], mop)
            sqp = trans.tile([P, 2 * nb, DL, W], bf16, tag=f"sqp{nb}")
            nc.scalar.activation(sqp[:], m1p[:], AF.Square)
            up = upool.tile([P, 2 * nb, DL, W], bf16, tag=f"up{nb}")
            nc.scalar.activation(up[:], sqp[:], AF.Exp,
                                 bias=bias_t[r2][:], scale=-0.5)
            return up

        def prods_accum(batch, up):
            nonlocal tcount
            for bi, pi in enumerate(batch):
                i, j, k = PAIRS[pi]
                for fr, sgn in ((0, 1), (1, -1)):
                    st, sp = (tcount == 0), (tcount == NFR - 1)
                    tcount += 1
                    si, sj, sk = sgn * i, sgn * j, sgn * k
                    mview = msk_h[sj][:, 0:3, 1 + si:1 + si + DL,
                                      1 + sk:1 + sk + W]
                    uf = up[:, 2 * bi + fr:2 * bi + fr + 1]
                    ub = uf.broadcast_to((P, 3, DL, W))
                    prods = trans.tile([P, 3, DL, W], bf16, tag="prods")
                    TT(prods[:], ub, mview, ALU.mult)
                    for ci in range(3):
                        nc.tensor.matmul(accP[:, ci], eye, prods[:, ci],
                                         start=st, stop=sp)
                    nc.tensor.matmul(accP[:, 3], eye, up[:, 2 * bi + fr],
                                     start=st, stop=sp)

        # Software-pipelined emission: PE subs lead their batch by one,
        # prods lag their batch by one, so every engine's program stays
        # stocked with ready work (no cross-engine head-of-line stalls).
        if not PESUBS_EARLY:
            pe_subs_for(BATCHES[0])
        pend = None  # (batch, up) awaiting prods
        for idx, batch in enumerate(BATCHES):
            if not PESUBS_EARLY and idx + 1 < len(BATCHES):
                pe_subs_for(BATCHES[idx + 1])
            up = trees_sq_exp(batch)
            if idx == 0:
                e2 = cpool.tile([P, 2, DL, W], bf16, tag="e2")
                TT(e2[:], expx[:, 0:2], expx[:, 2:4], ALU.add)
                esum = cpool.tile([P, DL, W], bf16, tag="esum")
                TT(esum[:], e2[:, 0], e2[:, 1], ALU.add)
            if pend is not None:
                prods_accum(*pend)
            pend = (batch, up)
        prods_accum(*pend)

        # ---- y-path (engine via env; Pool couples Pool->DVE mid-program) ----
        _ypath = nc.gpsimd if os.environ.get("V2_YPOOL", "1") == "1" \
            else nc.vector
        ym4 = cpool.tile([P, 4, DL, W], bf16, tag="ym4")
        _ypath.tensor_tensor(ym4[:], cv(masks, 0, 0), dx4[:], ALU.mult)
        v1 = cpool.tile([P, 2, DL, W], bf16, tag="v1")
        _ypath.tensor_tensor(v1[:], ym4[:, 0:2], ym4[:, 2:4], ALU.add)
        yj = cpool.tile([P, DL, W], bf16, tag="yj")
        nc.vector.scalar_tensor_tensor(yj[:], v1[:, 0], 0.0, v1[:, 1],
                                       ALU.add, ALU.add,
                                       accum_out=part[:, 2:3])

        # ---- readout + T partial ----
        acc4 = cpool.tile([P, 4, DL, W], bf16, tag="acc4")
        nc.scalar.copy(acc4[:], accP[:])
        rS = cpool.tile([P, DL, W],
                        bf16 if os.environ.get("V2_RSBF", "0") == "1" else f32,
                        tag="rS")
        if RECIP_PSUM:
            nc.vector.reciprocal_approx_fast(rS[:], accP[:, 3])
        else:
            Sf = cpool.tile([P, DL, W], f32, tag="Sf")
            nc.scalar.copy(Sf[:], accP[:, 3])
            nc.vector.reciprocal_approx_fast(rS[:], Sf[:])

        lse = cpool.tile([P, DL, W], bf16, tag="lse")
        if LSE_ACCUM:
            nc.scalar.activation(lse[:], esum[:], AF.Ln,
                                 accum_out=part[:, 0:1])
        else:
            nc.scalar.activation(lse[:], esum[:], AF.Ln)
            wjl = cpool.tile([P, DL, W], bf16, tag="wjl")
            nc.vector.scalar_tensor_tensor(
                wjl[:], lse[:], 0.0, lse[:], ALU.mult, ALU.add,
                accum_out=part[:, 0:1])

        tp4 = cpool.tile([P, 4, DL, W], bf16, tag="tp4")
        TT(tp4[:], acc4[:], dx4[:], ALU.mult)
        u1 = cpool.tile([P, 2, DL, W], bf16, tag="u1")
        TT(u1[:], tp4[:, 0:2], tp4[:, 2:4], ALU.add)
        Tt = cpool.tile([P, DL, W], bf16, tag="Tt")
        TT(Tt[:], u1[:, 0], u1[:, 1], ALU.add)
        wj = cpool.tile([P, DL, W], bf16, tag="wj")
        if USE_TTR:
            nc.vector.tensor_tensor_reduce(wj[:], Tt[:], rS[:], -0.5, 0.0,
                                           ALU.mult, ALU.add,
                                           accum_out=part[:, 1:2])
        else:
            w1 = cpool.tile([P, DL, W], f32, tag="w1")
            TT(w1[:], Tt[:], rS[:], ALU.mult)
            nc.vector.tensor_scalar(wj[:], w1[:], -0.5, 0.0, ALU.mult,
                                    ALU.add, accum_out=part[:, 1:2])
        nc.sync.dma_start(out_d[:, :], part[:])

    nc.compile()
    return nc


def _get_nc():
    if "nc" not in _CACHED:
        _CACHED["nc"] = _build_nc()
    return _CACHED["nc"]


def make_in_maps(inputs, labels, images):
    """Host-side shard + layout prep: (b,h)->partition transpose, d/w halo
    padding, h-shifted copies (clamped), one-hot masks w/ ones channel,
    bf16 pre-cast."""
    import ml_dtypes

    bf = ml_dtypes.bfloat16
    img = np.asarray(images, np.float32)
    lab = np.asarray(labels).astype(np.int32)
    lgt = np.asarray(inputs, np.float32)

    hp = np.minimum(np.arange(H) + 1, H - 1)
    hm = np.maximum(np.arange(H) - 1, 0)

    img_p = np.pad(img, ((0, 0), (0, 0), (1, 1), (0, 0), (1, 1)),
                   mode="edge").astype(bf)                    # [B,C,D+2,H,W+2]
    lab_p = np.pad(lab, ((0, 0), (1, 1), (0, 0), (1, 1)), mode="edge")
    msk_p = np.empty((B, 4, D + 2, H, W + 2), dtype=bf)       # ones ch at [3]
    for c in (1, 2, 3):
        msk_p[:, c - 1] = (lab_p == c).astype(bf)
    msk_p[:, 3] = bf(1.0)

    def shard(arr, k, hidx=None):
        a = arr[:, :, k * DL:k * DL + DE]
        if hidx is not None:
            a = a[:, :, :, hidx]
        return np.ascontiguousarray(
            a.transpose(0, 3, 1, 2, 4)).reshape(P, -1)

    lgtb = lgt.astype(bf)
    in_maps = []
    for k in range(NCORES):
        xm = np.ascontiguousarray(
            lgtb[:, :, k * DL:(k + 1) * DL].transpose(0, 3, 1, 2, 4)
        ).reshape(P, -1)
        in_maps.append({
            "imgc": shard(img_p, k),
            "imghp": shard(img_p, k, hp),
            "imghm": shard(img_p, k, hm),
            "mskc": shard(msk_p, k),
            "mskhp": shard(msk_p[:, 0:3], k, hp),
            "mskhm": shard(msk_p[:, 0:3], k, hm),
            "logits": xm,
            "eye": _mats(),
        })
    return in_maps


def _mats():
    """[I, -I] as one [P, 2P] bf16 array."""
    import ml_dtypes

    eye = np.eye(P, dtype=np.float32)
    out = np.concatenate([eye, -eye], axis=1)
    return np.ascontiguousarray(out).astype(ml_dtypes.bfloat16)


def kernel(inputs, labels, images):
    from concourse.bass_utils import run_bass_kernel_spmd

    nc = _get_nc()
    in_maps = make_in_maps(inputs, labels, images)
    res = run_bass_kernel_spmd(nc, in_maps, core_ids=list(range(NCORES)))
    total = 0.0
    for k in range(NCORES):
        p = res.results[k]["out"].astype(np.float64)
        total += p[:, 0].sum() + p[:, 1].sum() - 0.5 * p[:, 2].sum()
    return np.float32(total / NVOX)


# revision 7
# speedup vs baseline: 1.0084x; 1.0084x over previous
"""Trainium2 Bass kernel v2 for nn_CELossWithSVLS_VE (SVLS cross-entropy).

Math (as v1): per ±offset-pair trick with
  u_n = exp(-0.5*(maxdiff_n^2 + r_n^2)),  w_center = 1/2, w_n = u_n/(2S),
  loss(v) = lse(v) - 0.5*y(v) - 0.5*rS(v)*T(v).

v2 engine plan (cost-model driven; neuronxcc-validated op support):
- Host precomputes h-shifted images AND one-hot masks (4th channel == 1),
  all bf16: no on-device is_equal, no SBUF-shift DMAs, no Sh matrices.
- PHASE-MAJOR schedule: all channel-difference subs are emitted first
  into persistent tiles (PE via eye/negI matmul pairs + Act copybacks;
  Pool via gpsimd TT-subtract, its only fast ALU ops being add/sub/mult),
  so no engine head-of-line blocks another mid-pipeline.
- DVE (critical engine; all ops bf16 packed SBUF = 2x mode) keeps the
  max/min trees, mask products, and final combine; the logits phase is
  emitted first so DVE/Act start before the subs land.
- Act: squares+exps batched per equal-r2 pair group, lse, copybacks,
  PSUM readout.
- Final: accP[4] = [acc1..3, S]; dx4 = [dx1..3, x0]; T = sum(acc4*dx4);
  per-partition partials via accum_out (lse / -T/(2S) / y) -> host sum.
"""
import os
import sys
from contextlib import ExitStack

import numpy as np

if "/opt/trn_rl_repo" not in sys.path:
    sys.path.insert(0, "/opt/trn_rl_repo")

B, C, D, H, W = 2, 4, 64, 64, 64
NCORES = 8
DL = D // NCORES          # 8 local d-planes
DE, WE = DL + 2, W + 2    # 10, 66 (d/w halos)
P = 128                   # partitions = (b, h)
NVOX = B * D * H * W      # 524288

# 13 positive offsets; r2 = i*i+j*j+k*k sets the exp bias.
# Indices 0-3: j==0 (extended-box shared sub); 4-12: j!=0.
PAIRS = [
    (1, 0, 0), (0, 0, 1), (1, 0, 1), (1, 0, -1),
    (0, 1, 0), (1, 1, 0), (1, -1, 0), (0, 1, 1), (0, 1, -1),
    (1, 1, 1), (1, 1, -1), (1, -1, 1), (1, -1, -1),
]

# Engine assignment (pair indices).
def _envset(name, default):
    raw = os.environ.get(name)
    if raw is None:
        return frozenset(default)
    return frozenset(int(v) for v in raw.split(",") if v != "")


PE_MAX = frozenset(range(4, 13))      # j!=0 max-frame subs on PE
PE_MIN = _envset("V2_PE_MIN", {6})               # min-frame subs on PE
POOL_J0 = _envset("V2_POOL_J0", {0, 1})          # j==0 ext-box subs on Pool
POOL_MIN = frozenset(range(4, 13)) - PE_MIN      # rest of min-subs on Pool
# Consumer order: PE-sub pairs first (ready earliest), then Pool pairs in
# Pool program order.  Batches group consecutive equal-r2 pairs for shared
# square+exp activations.
_BATCH_DEF = "4|5,6|2,3|0,1|7,8|9,10|11,12"
BATCHES = [[int(v) for v in grp.split(",")]
           for grp in os.environ.get("V2_BATCHES", _BATCH_DEF).split("|")]

LSE_ACCUM = os.environ.get("V2_LSE_ACCUM", "0") == "1"
USE_TTR = os.environ.get("V2_TTR", "0") == "1"
RECIP_PSUM = os.environ.get("V2_RECIP_PSUM", "1") == "1"
TAB6 = os.environ.get("V2_TAB6", "1") == "1"

_CACHED = {}


def _build_nc():
    import concourse.bacc as bacc
    import concourse.mybir as mybir
    import concourse.tile as tile

    AF = mybir.ActivationFunctionType
    ALU = mybir.AluOpType
    dt = mybir.dt

    nc = bacc.Bacc("TRN2", target_bir_lowering=False, debug=False,
                   num_devices=NCORES)
    imgc_d = nc.dram_tensor("imgc", [P, C * DE * WE], dt.bfloat16,
                            kind="ExternalInput")
    imghp_d = nc.dram_tensor("imghp", [P, C * DE * WE], dt.bfloat16,
                             kind="ExternalInput")
    imghm_d = nc.dram_tensor("imghm", [P, C * DE * WE], dt.bfloat16,
                             kind="ExternalInput")
    mskc_d = nc.dram_tensor("mskc", [P, 4 * DE * WE], dt.bfloat16,
                            kind="ExternalInput")
    mskhp_d = nc.dram_tensor("mskhp", [P, 3 * DE * WE], dt.bfloat16,
                             kind="ExternalInput")
    mskhm_d = nc.dram_tensor("mskhm", [P, 3 * DE * WE], dt.bfloat16,
                             kind="ExternalInput")
    logit_d = nc.dram_tensor("logits", [P, C * DL * W], dt.bfloat16,
                             kind="ExternalInput")
    eye_d = nc.dram_tensor("eye", [P, 2 * P], dt.bfloat16,
                           kind="ExternalInput")
    out_d = nc.dram_tensor("out", [P, 3], dt.float32, kind="ExternalOutput")

    import concourse.bass as bass_mod

    with tile.TileContext(nc) as tc, ExitStack() as ctx:
        persist = ctx.enter_context(tc.tile_pool(name="persist", bufs=1))
        subsp = ctx.enter_context(tc.tile_pool(name="subsp", bufs=1))
        cpool = ctx.enter_context(tc.tile_pool(name="cpool", bufs=1))
        trans = ctx.enter_context(tc.tile_pool(name="trans", bufs=int(os.environ.get("V2_TRANS", "2"))))
        upool = ctx.enter_context(tc.tile_pool(name="upool", bufs=int(os.environ.get("V2_UPOOL", "2"))))
        psum = ctx.enter_context(
            tc.tile_pool(name="psum", bufs=1, space=bass_mod.MemorySpace.PSUM))
        psum2 = ctx.enter_context(
            tc.tile_pool(name="psum2", bufs=2, space=bass_mod.MemorySpace.PSUM))
        dhpool = ctx.enter_context(
            tc.tile_pool(name="dhpool",
                         bufs=int(os.environ.get("V2_DHBUFS", "3"))))

        f32, bf16 = dt.float32, dt.bfloat16
        TT = nc.vector.tensor_tensor

        if TAB6:
            nc.scalar.add_instruction(mybir.InstLoadActFuncSet(
                name=nc.get_next_instruction_name(), act_func_set_id=6,
                ins=[], outs=[]))

        # ---- loads.  SP queue carries the critical-path-early tensors in
        # priority order; DVE/Act queues carry the h-shifted copies so the
        # three HWDGE queues fill SBUF in parallel. ----
        EARLY = os.environ.get("V2_EARLY", "0") == "1"
        IMGFIRST = os.environ.get("V2_IMGFIRST", "0") == "1"
        imgb = persist.tile([P, C, DE, WE], bf16, tag="imgb")
        if IMGFIRST:
            nc.sync.dma_start(imgb[:], imgc_d[:, :])
        x = persist.tile([P, C, DL, W], bf16, tag="x")
        if EARLY:
            nc.sync.dma_start(x[:, 0:2], logit_d[:, 0:2 * DL * W])
            nc.sync.dma_start(x[:, 2:4], logit_d[:, 2 * DL * W:4 * DL * W])
        else:
            nc.sync.dma_start(x[:], logit_d[:, :])
        if not IMGFIRST:
            nc.sync.dma_start(imgb[:], imgc_d[:, :])
        mats = persist.tile([P, 2, P], bf16, tag="mats")
        nc.sync.dma_start(mats[:], eye_d[:, :])
        dx4 = persist.tile([P, 4, DL, W], bf16, tag="dx4")
        nc.sync.dma_start(dx4[:, 3], logit_d[:, 0:DL * W])  # x0 channel
        imgb_hp = persist.tile([P, C, DE, WE], bf16, tag="imgb_hp")
        nc.scalar.dma_start(imgb_hp[:], imghp_d[:, :])
        imgb_hm = persist.tile([P, C, DE, WE], bf16, tag="imgb_hm")
        nc.scalar.dma_start(imgb_hm[:], imghm_d[:, :])
        masks = persist.tile([P, 4, DE, WE], bf16, tag="masks")
        (nc.scalar if EARLY else nc.sync).dma_start(masks[:], mskc_d[:, :])
        masks_hp = persist.tile([P, 3, DE, WE], bf16, tag="masks_hp")
        nc.scalar.dma_start(masks_hp[:], mskhp_d[:, :])
        masks_hm = persist.tile([P, 3, DE, WE], bf16, tag="masks_hm")
        nc.scalar.dma_start(masks_hm[:], mskhm_d[:, :])

        img_h = {1: imgb_hp, 0: imgb, -1: imgb_hm}
        msk_h = {1: masks_hp, 0: masks, -1: masks_hm}
        eye, negI = mats[:, 0], mats[:, 1]

        def cv(tile_, i, k, ch=None):
            """center view shifted by (i, ., k) of a [..., DE, WE] tile."""
            if ch is None:
                return tile_[:, :, 1 + i:1 + i + DL, 1 + k:1 + k + W]
            return tile_[:, ch, 1 + i:1 + i + DL, 1 + k:1 + k + W]

        bias_t = {}
        for r2 in (1.0, 2.0, 3.0):
            bt = persist.tile([P, 1], f32, tag=f"bias{int(r2)}")
            nc.gpsimd.memset(bt[:], -0.5 * r2)
            bias_t[r2] = bt

        # ---- warm-up phase: logits-only work (x DMA lands first) ----
        WARMUP_MARK = True
        part = cpool.tile([P, 3], f32, tag="part")
        expx = cpool.tile([P, C, DL, W], bf16, tag="expx")
        if EARLY:
            nc.scalar.activation(expx[:, 0:2], x[:, 0:2], AF.Exp)
            nc.scalar.activation(expx[:, 2:4], x[:, 2:4], AF.Exp)
        else:
            nc.scalar.activation(expx[:], x[:], AF.Exp)
        TT(dx4[:, 0:3], x[:, 1:4],
           x[:, 0:1].broadcast_to((P, 3, DL, W)), ALU.subtract)

        # ---- sub phase: all channel-difference arrays, PE + Pool ----
        subt = {}  # (pi, fr) -> ('halves', [hhA, hhB]) | ('view', fn)

        def pe_sub(pi, fr, pos_view, neg_view):
            d4h = []
            for half in range(2):
                d4p = psum2.tile([P, 2, DL, W], f32, tag="d4p")
                for cc in range(2):
                    c = 2 * half + cc
                    nc.tensor.matmul(d4p[:, cc], eye, pos_view(c),
                                     start=True, stop=False)
                    nc.tensor.matmul(d4p[:, cc], negI, neg_view(c),
                                     start=False, stop=True)
                if os.environ.get("V2_D4HRING", "0") == "1":
                    hh = dhpool.tile([P, 2, DL, W], bf16,
                                     tag=f"d4h{half}")
                else:
                    hh = subsp.tile([P, 2, DL, W], bf16,
                                    tag=f"d4h_{pi}_{fr}_{half}")
                nc.scalar.copy(hh[:], d4p[:])
                d4h.append(hh)
            subt[(pi, fr)] = ("halves", d4h)

        def pe_subs_for(batch):
            for pi in batch:
                i, j, k = PAIRS[pi]
                if pi in PE_MAX:
                    pe_sub(pi, 0,
                           lambda c, j=j, i=i, k=k: cv(img_h[j], i, k, c),
                           lambda c: cv(imgb, 0, 0, c))
                if pi in PE_MIN:
                    pe_sub(pi, 1, lambda c: cv(imgb, 0, 0, c),
                           lambda c, j=j, i=i, k=k: cv(img_h[-j], -i, -k, c))

        # Subs on Pool get a channel split (Pool writes [0:SPLIT), DVE the
        # rest into the same tile) so Pool's serial per-sub latency shrinks;
        # the tree's first level consumes both halves naturally.
        SPLIT = int(os.environ.get("V2_SPLIT", "4"))

        def split_sub(dst, pos, neg, on_pool):
            if on_pool and SPLIT < C:
                nc.gpsimd.tensor_tensor(dst[:, 0:SPLIT], pos[:, 0:SPLIT],
                                        neg[:, 0:SPLIT], ALU.subtract)
                TT(dst[:, SPLIT:C], pos[:, SPLIT:C], neg[:, SPLIT:C],
                   ALU.subtract)
            else:
                eng = nc.gpsimd if on_pool else nc.vector
                eng.tensor_tensor(dst[:], pos[:], neg[:], ALU.subtract)

        PESUBS_EARLY = os.environ.get("V2_PESUBS_EARLY", "0") == "1"
        if PESUBS_EARLY:
            pe_need = sorted(set(PE_MAX) | set(PE_MIN))
            pe_subs_for(pe_need)

        pool_order = [pi for b in BATCHES for pi in b
                      if pi in POOL_J0 or pi in POOL_MIN or
                      (pi < 4 and pi not in POOL_J0)]
        for pi in pool_order:
            i, j, k = PAIRS[pi]
            if pi < 4:
                nd, nw = (9 if i else 8), (65 if k else 64)
                d0, w0 = (0 if i == 1 else 1), (0 if k == 1 else 1)
                dpe = subsp.tile([P, C, nd, nw], bf16, tag=f"dpe_{pi}")
                split_sub(dpe,
                          imgb[:, :, d0 + i:d0 + i + nd, w0 + k:w0 + k + nw],
                          imgb[:, :, d0:d0 + nd, w0:w0 + nw], pi in POOL_J0)

                def j0_view(fr, dpe=dpe, d0=d0, w0=w0, i=i, k=k):
                    ds = 1 - d0 - (i if fr else 0)
                    ws = 1 - w0 - (k if fr else 0)
                    return dpe[:, :, ds:ds + DL, ws:ws + W]
                subt[(pi, 0)] = ("view", j0_view(0))
                subt[(pi, 1)] = ("view", j0_view(1))
            else:
                d4m = subsp.tile([P, C, DL, W], bf16, tag=f"d4m_{pi}")
                split_sub(d4m, cv(imgb, 0, 0), cv(img_h[-j], -i, -k), True)
                subt[(pi, 1)] = ("view", d4m[:])

        # ---- consumer loop: trees -> batched sq+exp -> prods+accum ----
        accP = psum.tile([P, 4, DL, W], f32, tag="accP")
        NFR = 2 * len(PAIRS)

        def tree(dst, hi, lo, op):
            m2 = trans.tile([P, 2, DL, W], bf16, tag="m2")
            TT(m2[:], hi, lo, op)
            TT(dst, m2[:, 0], m2[:, 1], op)

        tcount = 0

        def trees_sq_exp(batch):
            nb = len(batch)
            r2 = float(sum(v * v for v in PAIRS[batch[0]]))
            m1p = trans.tile([P, 2 * nb, DL, W], bf16, tag=f"m1p{nb}")
            for bi, pi in enumerate(batch):
                for fr in range(2):
                    mop = ALU.max if fr == 0 else ALU.min
                    kind, val = subt[(pi, fr)]
                    dst = m1p[:, 2 * bi + fr]
                    if kind == "halves":
                        tree(dst, val[0][:], val[1][:], mop)
                    else:
                        tree(dst, val[:, 0:2], val[:, 2:4], mop)
            sqp = trans.tile([P, 2 * nb, DL, W], bf16, tag=f"sqp{nb}")
            nc.scalar.activation(sqp[:], m1p[:], AF.Square)
            up = upool.tile([P, 2 * nb, DL, W], bf16, tag=f"up{nb}")
            nc.scalar.activation(up[:], sqp[:], AF.Exp,
                                 bias=bias_t[r2][:], scale=-0.5)
            return up

        def prods_accum(batch, up):
            nonlocal tcount
            for bi, pi in enumerate(batch):
                i, j, k = PAIRS[pi]
                for fr, sgn in ((0, 1), (1, -1)):
                    st, sp = (tcount == 0), (tcount == NFR - 1)
                    tcount += 1
                    si, sj, sk = sgn * i, sgn * j, sgn * k
                    mview = msk_h[sj][:, 0:3, 1 + si:1 + si + DL,
                                      1 + sk:1 + sk + W]
                    uf = up[:, 2 * bi + fr:2 * bi + fr + 1]
                    ub = uf.broadcast_to((P, 3, DL, W))
                    prods = trans.tile([P, 3, DL, W], bf16, tag="prods")
                    TT(prods[:], ub, mview, ALU.mult)
                    for ci in range(3):
                        nc.tensor.matmul(accP[:, ci], eye, prods[:, ci],
                                         start=st, stop=sp)
                    nc.tensor.matmul(accP[:, 3], eye, up[:, 2 * bi + fr],
                                     start=st, stop=sp)

        # Software-pipelined emission: PE subs lead their batch by one,
        # prods lag their batch by one, so every engine's program stays
        # stocked with ready work (no cross-engine head-of-line stalls).
        if not PESUBS_EARLY:
            pe_subs_for(BATCHES[0])
        pend = None  # (batch, up) awaiting prods
        for idx, batch in enumerate(BATCHES):
            if not PESUBS_EARLY and idx + 1 < len(BATCHES):
                pe_subs_for(BATCHES[idx + 1])
            up = trees_sq_exp(batch)
            if idx == 0:
                e2 = cpool.tile([P, 2, DL, W], bf16, tag="e2")
                TT(e2[:], expx[:, 0:2], expx[:, 2:4], ALU.add)
                esum = cpool.tile([P, DL, W], bf16, tag="esum")
                TT(esum[:], e2[:, 0], e2[:, 1], ALU.add)
            if pend is not None:
                prods_accum(*pend)
            pend = (batch, up)
        prods_accum(*pend)

        # ---- y-path (engine via env; Pool couples Pool->DVE mid-program) ----
        _ypath = nc.gpsimd if os.environ.get("V2_YPOOL", "1") == "1" \
            else nc.vector
        ym4 = cpool.tile([P, 4, DL, W], bf16, tag="ym4")
        _ypath.tensor_tensor(ym4[:], cv(masks, 0, 0), dx4[:], ALU.mult)
        v1 = cpool.tile([P, 2, DL, W], bf16, tag="v1")
        _ypath.tensor_tensor(v1[:], ym4[:, 0:2], ym4[:, 2:4], ALU.add)
        yj = cpool.tile([P, DL, W], bf16, tag="yj")
        nc.vector.scalar_tensor_tensor(yj[:], v1[:, 0], 0.0, v1[:, 1],
                                       ALU.add, ALU.add,
                                       accum_out=part[:, 2:3])

        # ---- readout + T partial ----
        TP4PSUM = os.environ.get("V2_TP4PSUM", "1") == "1"
        if not TP4PSUM:
            acc4 = cpool.tile([P, 4, DL, W], bf16, tag="acc4")
            nc.scalar.copy(acc4[:], accP[:])
        rS = cpool.tile([P, DL, W],
                        bf16 if os.environ.get("V2_RSBF", "0") == "1" else f32,
                        tag="rS")
        if RECIP_PSUM:
            nc.vector.reciprocal_approx_fast(rS[:], accP[:, 3])
        else:
            Sf = cpool.tile([P, DL, W], f32, tag="Sf")
            nc.scalar.copy(Sf[:], accP[:, 3])
            nc.vector.reciprocal_approx_fast(rS[:], Sf[:])

        lse = cpool.tile([P, DL, W], bf16, tag="lse")
        if LSE_ACCUM:
            nc.scalar.activation(lse[:], esum[:], AF.Ln,
                                 accum_out=part[:, 0:1])
        else:
            nc.scalar.activation(lse[:], esum[:], AF.Ln)
            wjl = cpool.tile([P, DL, W], bf16, tag="wjl")
            nc.vector.scalar_tensor_tensor(
                wjl[:], lse[:], 0.0, lse[:], ALU.mult, ALU.add,
                accum_out=part[:, 0:1])

        tp4 = cpool.tile([P, 4, DL, W], bf16, tag="tp4")
        TT(tp4[:], accP[:] if TP4PSUM else acc4[:], dx4[:], ALU.mult)
        u1 = cpool.tile([P, 2, DL, W], bf16, tag="u1")
        TT(u1[:], tp4[:, 0:2], tp4[:, 2:4], ALU.add)
        Tt = cpool.tile([P, DL, W], bf16, tag="Tt")
        TT(Tt[:], u1[:, 0], u1[:, 1], ALU.add)
        wj = cpool.tile([P, DL, W], bf16, tag="wj")
        if USE_TTR:
            nc.vector.tensor_tensor_reduce(wj[:], Tt[:], rS[:], -0.5, 0.0,
                                           ALU.mult, ALU.add,
                                           accum_out=part[:, 1:2])
        else:
            w1 = cpool.tile([P, DL, W], f32, tag="w1")
            TT(w1[:], Tt[:], rS[:], ALU.mult)
            nc.vector.tensor_scalar(wj[:], w1[:], -0.5, 0.0, ALU.mult,
                                    ALU.add, accum_out=part[:, 1:2])
        nc.sync.dma_start(out_d[:, :], part[:])

    nc.compile()
    return nc


def _get_nc():
    if "nc" not in _CACHED:
        _CACHED["nc"] = _build_nc()
    return _CACHED["nc"]


def make_in_maps(inputs, labels, images):
    """Host-side shard + layout prep: (b,h)->partition transpose, d/w halo
    padding, h-shifted copies (clamped), one-hot masks w/ ones channel,
    bf16 pre-cast."""
    import ml_dtypes

    bf = ml_dtypes.bfloat16
    img = np.asarray(images, np.float32)
    lab = np.asarray(labels).astype(np.int32)
    lgt = np.asarray(inputs, np.float32)

    hp = np.minimum(np.arange(H) + 1, H - 1)
    hm = np.maximum(np.arange(H) - 1, 0)

    img_p = np.pad(img, ((0, 0), (0, 0), (1, 1), (0, 0), (1, 1)),
                   mode="edge").astype(bf)                    # [B,C,D+2,H,W+2]
    lab_p = np.pad(lab, ((0, 0), (1, 1), (0, 0), (1, 1)), mode="edge")
    msk_p = np.empty((B, 4, D + 2, H, W + 2), dtype=bf)       # ones ch at [3]
    for c in (1, 2, 3):
        msk_p[:, c - 1] = (lab_p == c).astype(bf)
    msk_p[:, 3] = bf(1.0)

    def shard(arr, k, hidx=None):
        a = arr[:, :, k * DL:k * DL + DE]
        if hidx is not None:
            a = a[:, :, :, hidx]
        return np.ascontiguousarray(
            a.transpose(0, 3, 1, 2, 4)).reshape(P, -1)

    lgtb = lgt.astype(bf)
    in_maps = []
    for k in range(NCORES):
        xm = np.ascontiguousarray(
            lgtb[:, :, k * DL:(k + 1) * DL].transpose(0, 3, 1, 2, 4)
        ).reshape(P, -1)
        in_maps.append({
            "imgc": shard(img_p, k),
            "imghp": shard(img_p, k, hp),
            "imghm": shard(img_p, k, hm),
            "mskc": shard(msk_p, k),
            "mskhp": shard(msk_p[:, 0:3], k, hp),
            "mskhm": shard(msk_p[:, 0:3], k, hm),
            "logits": xm,
            "eye": _mats(),
        })
    return in_maps


def _mats():
    """[I, -I] as one [P, 2P] bf16 array."""
    import ml_dtypes

    eye = np.eye(P, dtype=np.float32)
    out = np.concatenate([eye, -eye], axis=1)
    return np.ascontiguousarray(out).astype(ml_dtypes.bfloat16)


def kernel(inputs, labels, images):
    from concourse.bass_utils import run_bass_kernel_spmd

    nc = _get_nc()
    in_maps = make_in_maps(inputs, labels, images)
    res = run_bass_kernel_spmd(nc, in_maps, core_ids=list(range(NCORES)))
    total = 0.0
    for k in range(NCORES):
        p = res.results[k]["out"].astype(np.float64)
        total += p[:, 0].sum() + p[:, 1].sum() - 0.5 * p[:, 2].sum()
    return np.float32(total / NVOX)


# revision 8
# speedup vs baseline: 1.0351x; 1.0264x over previous
"""Trainium2 Bass kernel v2 for nn_CELossWithSVLS_VE (SVLS cross-entropy).

Math (as v1): per ±offset-pair trick with
  u_n = exp(-0.5*(maxdiff_n^2 + r_n^2)),  w_center = 1/2, w_n = u_n/(2S),
  loss(v) = lse(v) - 0.5*y(v) - 0.5*rS(v)*T(v).

v2 engine plan (cost-model driven; neuronxcc-validated op support):
- Host precomputes h-shifted images AND one-hot masks (4th channel == 1),
  all bf16: no on-device is_equal, no SBUF-shift DMAs, no Sh matrices.
- PHASE-MAJOR schedule: all channel-difference subs are emitted first
  into persistent tiles (PE via eye/negI matmul pairs + Act copybacks;
  Pool via gpsimd TT-subtract, its only fast ALU ops being add/sub/mult),
  so no engine head-of-line blocks another mid-pipeline.
- DVE (critical engine; all ops bf16 packed SBUF = 2x mode) keeps the
  max/min trees, mask products, and final combine; the logits phase is
  emitted first so DVE/Act start before the subs land.
- Act: squares+exps batched per equal-r2 pair group, lse, copybacks,
  PSUM readout.
- Final: accP[4] = [acc1..3, S]; dx4 = [dx1..3, x0]; T = sum(acc4*dx4);
  per-partition partials via accum_out (lse / -T/(2S) / y) -> host sum.
"""
import os
import sys
from contextlib import ExitStack

import numpy as np

if "/opt/trn_rl_repo" not in sys.path:
    sys.path.insert(0, "/opt/trn_rl_repo")

B, C, D, H, W = 2, 4, 64, 64, 64
NCORES = 8
DL = D // NCORES          # 8 local d-planes
DE, WE = DL + 2, W + 2    # 10, 66 (d/w halos)
P = 128                   # partitions = (b, h)
NVOX = B * D * H * W      # 524288

# 13 positive offsets; r2 = i*i+j*j+k*k sets the exp bias.
# Indices 0-3: j==0 (extended-box shared sub); 4-12: j!=0.
PAIRS = [
    (1, 0, 0), (0, 0, 1), (1, 0, 1), (1, 0, -1),
    (0, 1, 0), (1, 1, 0), (1, -1, 0), (0, 1, 1), (0, 1, -1),
    (1, 1, 1), (1, 1, -1), (1, -1, 1), (1, -1, -1),
]

# Engine assignment (pair indices).
def _envset(name, default):
    raw = os.environ.get(name)
    if raw is None:
        return frozenset(default)
    return frozenset(int(v) for v in raw.split(",") if v != "")


PE_MAX = frozenset(range(4, 13))      # j!=0 max-frame subs on PE
PE_MIN = _envset("V2_PE_MIN", {6})               # min-frame subs on PE
POOL_J0 = _envset("V2_POOL_J0", {0, 1})          # j==0 ext-box subs on Pool
POOL_MIN = frozenset(range(4, 13)) - PE_MIN      # rest of min-subs on Pool
# Consumer order: PE-sub pairs first (ready earliest), then Pool pairs in
# Pool program order.  Batches group consecutive equal-r2 pairs for shared
# square+exp activations.
_BATCH_DEF = "5,6|2,3|0,1|7,8|9,10|11,12|4"
BATCHES = [[int(v) for v in grp.split(",")]
           for grp in os.environ.get("V2_BATCHES", _BATCH_DEF).split("|")]

LSE_ACCUM = os.environ.get("V2_LSE_ACCUM", "0") == "1"
USE_TTR = os.environ.get("V2_TTR", "0") == "1"
RECIP_PSUM = os.environ.get("V2_RECIP_PSUM", "1") == "1"
TAB6 = os.environ.get("V2_TAB6", "1") == "1"

_CACHED = {}


def _build_nc():
    import concourse.bacc as bacc
    import concourse.mybir as mybir
    import concourse.tile as tile

    AF = mybir.ActivationFunctionType
    ALU = mybir.AluOpType
    dt = mybir.dt

    nc = bacc.Bacc("TRN2", target_bir_lowering=False, debug=False,
                   num_devices=NCORES)
    imgc_d = nc.dram_tensor("imgc", [P, C * DE * WE], dt.bfloat16,
                            kind="ExternalInput")
    imghp_d = nc.dram_tensor("imghp", [P, C * DE * WE], dt.bfloat16,
                             kind="ExternalInput")
    imghm_d = nc.dram_tensor("imghm", [P, C * DE * WE], dt.bfloat16,
                             kind="ExternalInput")
    mskc_d = nc.dram_tensor("mskc", [P, 4 * DE * WE], dt.bfloat16,
                            kind="ExternalInput")
    mskhp_d = nc.dram_tensor("mskhp", [P, 3 * DE * WE], dt.bfloat16,
                             kind="ExternalInput")
    mskhm_d = nc.dram_tensor("mskhm", [P, 3 * DE * WE], dt.bfloat16,
                             kind="ExternalInput")
    logit_d = nc.dram_tensor("logits", [P, C * DL * W], dt.bfloat16,
                             kind="ExternalInput")
    eye_d = nc.dram_tensor("eye", [P, 2 * P], dt.bfloat16,
                           kind="ExternalInput")
    out_d = nc.dram_tensor("out", [P, 3], dt.float32, kind="ExternalOutput")

    import concourse.bass as bass_mod

    with tile.TileContext(nc) as tc, ExitStack() as ctx:
        persist = ctx.enter_context(tc.tile_pool(name="persist", bufs=1))
        subsp = ctx.enter_context(tc.tile_pool(name="subsp", bufs=1))
        cpool = ctx.enter_context(tc.tile_pool(name="cpool", bufs=1))
        trans = ctx.enter_context(tc.tile_pool(name="trans", bufs=int(os.environ.get("V2_TRANS", "2"))))
        upool = ctx.enter_context(tc.tile_pool(name="upool", bufs=int(os.environ.get("V2_UPOOL", "2"))))
        psum = ctx.enter_context(
            tc.tile_pool(name="psum", bufs=1, space=bass_mod.MemorySpace.PSUM))
        psum2 = ctx.enter_context(
            tc.tile_pool(name="psum2", bufs=2, space=bass_mod.MemorySpace.PSUM))
        dhpool = ctx.enter_context(
            tc.tile_pool(name="dhpool",
                         bufs=int(os.environ.get("V2_DHBUFS", "3"))))

        f32, bf16 = dt.float32, dt.bfloat16
        TT = nc.vector.tensor_tensor

        if TAB6:
            nc.scalar.add_instruction(mybir.InstLoadActFuncSet(
                name=nc.get_next_instruction_name(), act_func_set_id=6,
                ins=[], outs=[]))

        # ---- loads.  SP queue carries the critical-path-early tensors in
        # priority order; DVE/Act queues carry the h-shifted copies so the
        # three HWDGE queues fill SBUF in parallel. ----
        EARLY = os.environ.get("V2_EARLY", "0") == "1"
        IMGFIRST = os.environ.get("V2_IMGFIRST", "0") == "1"
        imgb = persist.tile([P, C, DE, WE], bf16, tag="imgb")
        if IMGFIRST:
            nc.sync.dma_start(imgb[:], imgc_d[:, :])
        x = persist.tile([P, C, DL, W], bf16, tag="x")
        if EARLY:
            nc.sync.dma_start(x[:, 0:2], logit_d[:, 0:2 * DL * W])
            nc.sync.dma_start(x[:, 2:4], logit_d[:, 2 * DL * W:4 * DL * W])
        else:
            nc.sync.dma_start(x[:], logit_d[:, :])
        if not IMGFIRST:
            nc.sync.dma_start(imgb[:], imgc_d[:, :])
        mats = persist.tile([P, 2, P], bf16, tag="mats")
        nc.sync.dma_start(mats[:], eye_d[:, :])
        dx4 = persist.tile([P, 4, DL, W], bf16, tag="dx4")
        nc.sync.dma_start(dx4[:, 3], logit_d[:, 0:DL * W])  # x0 channel
        imgb_hp = persist.tile([P, C, DE, WE], bf16, tag="imgb_hp")
        nc.scalar.dma_start(imgb_hp[:], imghp_d[:, :])
        imgb_hm = persist.tile([P, C, DE, WE], bf16, tag="imgb_hm")
        nc.scalar.dma_start(imgb_hm[:], imghm_d[:, :])
        masks = persist.tile([P, 4, DE, WE], bf16, tag="masks")
        (nc.scalar if EARLY else nc.sync).dma_start(masks[:], mskc_d[:, :])
        masks_hp = persist.tile([P, 3, DE, WE], bf16, tag="masks_hp")
        nc.scalar.dma_start(masks_hp[:], mskhp_d[:, :])
        masks_hm = persist.tile([P, 3, DE, WE], bf16, tag="masks_hm")
        nc.scalar.dma_start(masks_hm[:], mskhm_d[:, :])

        img_h = {1: imgb_hp, 0: imgb, -1: imgb_hm}
        msk_h = {1: masks_hp, 0: masks, -1: masks_hm}
        eye, negI = mats[:, 0], mats[:, 1]

        def cv(tile_, i, k, ch=None):
            """center view shifted by (i, ., k) of a [..., DE, WE] tile."""
            if ch is None:
                return tile_[:, :, 1 + i:1 + i + DL, 1 + k:1 + k + W]
            return tile_[:, ch, 1 + i:1 + i + DL, 1 + k:1 + k + W]

        bias_t = {}
        for r2 in (1.0, 2.0, 3.0):
            bt = persist.tile([P, 1], f32, tag=f"bias{int(r2)}")
            nc.gpsimd.memset(bt[:], -0.5 * r2)
            bias_t[r2] = bt

        # ---- warm-up phase: logits-only work (x DMA lands first) ----
        WARMUP_MARK = True
        part = cpool.tile([P, 3], f32, tag="part")
        expx = cpool.tile([P, C, DL, W], bf16, tag="expx")
        if EARLY:
            nc.scalar.activation(expx[:, 0:2], x[:, 0:2], AF.Exp)
            nc.scalar.activation(expx[:, 2:4], x[:, 2:4], AF.Exp)
        else:
            nc.scalar.activation(expx[:], x[:], AF.Exp)
        TT(dx4[:, 0:3], x[:, 1:4],
           x[:, 0:1].broadcast_to((P, 3, DL, W)), ALU.subtract)

        # ---- sub phase: all channel-difference arrays, PE + Pool ----
        subt = {}  # (pi, fr) -> ('halves', [hhA, hhB]) | ('view', fn)

        def pe_sub(pi, fr, pos_view, neg_view):
            d4h = []
            for half in range(2):
                d4p = psum2.tile([P, 2, DL, W], f32, tag="d4p")
                for cc in range(2):
                    c = 2 * half + cc
                    nc.tensor.matmul(d4p[:, cc], eye, pos_view(c),
                                     start=True, stop=False)
                    nc.tensor.matmul(d4p[:, cc], negI, neg_view(c),
                                     start=False, stop=True)
                if os.environ.get("V2_D4HRING", "0") == "1":
                    hh = dhpool.tile([P, 2, DL, W], bf16,
                                     tag=f"d4h{half}")
                else:
                    hh = subsp.tile([P, 2, DL, W], bf16,
                                    tag=f"d4h_{pi}_{fr}_{half}")
                nc.scalar.copy(hh[:], d4p[:])
                d4h.append(hh)
            subt[(pi, fr)] = ("halves", d4h)

        def pe_subs_for(batch):
            for pi in batch:
                i, j, k = PAIRS[pi]
                if pi in PE_MAX:
                    pe_sub(pi, 0,
                           lambda c, j=j, i=i, k=k: cv(img_h[j], i, k, c),
                           lambda c: cv(imgb, 0, 0, c))
                if pi in PE_MIN:
                    pe_sub(pi, 1, lambda c: cv(imgb, 0, 0, c),
                           lambda c, j=j, i=i, k=k: cv(img_h[-j], -i, -k, c))

        # Subs on Pool get a channel split (Pool writes [0:SPLIT), DVE the
        # rest into the same tile) so Pool's serial per-sub latency shrinks;
        # the tree's first level consumes both halves naturally.
        SPLIT = int(os.environ.get("V2_SPLIT", "4"))

        def split_sub(dst, pos, neg, on_pool):
            if on_pool and SPLIT < C:
                nc.gpsimd.tensor_tensor(dst[:, 0:SPLIT], pos[:, 0:SPLIT],
                                        neg[:, 0:SPLIT], ALU.subtract)
                TT(dst[:, SPLIT:C], pos[:, SPLIT:C], neg[:, SPLIT:C],
                   ALU.subtract)
            else:
                eng = nc.gpsimd if on_pool else nc.vector
                eng.tensor_tensor(dst[:], pos[:], neg[:], ALU.subtract)

        PESUBS_EARLY = os.environ.get("V2_PESUBS_EARLY", "0") == "1"
        if PESUBS_EARLY:
            pe_need = sorted(set(PE_MAX) | set(PE_MIN))
            pe_subs_for(pe_need)

        pool_order = [pi for b in BATCHES for pi in b
                      if pi in POOL_J0 or pi in POOL_MIN or
                      (pi < 4 and pi not in POOL_J0)]
        for pi in pool_order:
            i, j, k = PAIRS[pi]
            if pi < 4:
                nd, nw = (9 if i else 8), (65 if k else 64)
                d0, w0 = (0 if i == 1 else 1), (0 if k == 1 else 1)
                dpe = subsp.tile([P, C, nd, nw], bf16, tag=f"dpe_{pi}")
                split_sub(dpe,
                          imgb[:, :, d0 + i:d0 + i + nd, w0 + k:w0 + k + nw],
                          imgb[:, :, d0:d0 + nd, w0:w0 + nw], pi in POOL_J0)

                def j0_view(fr, dpe=dpe, d0=d0, w0=w0, i=i, k=k):
                    ds = 1 - d0 - (i if fr else 0)
                    ws = 1 - w0 - (k if fr else 0)
                    return dpe[:, :, ds:ds + DL, ws:ws + W]
                subt[(pi, 0)] = ("view", j0_view(0))
                subt[(pi, 1)] = ("view", j0_view(1))
            else:
                d4m = subsp.tile([P, C, DL, W], bf16, tag=f"d4m_{pi}")
                split_sub(d4m, cv(imgb, 0, 0), cv(img_h[-j], -i, -k), True)
                subt[(pi, 1)] = ("view", d4m[:])

        # ---- consumer loop: trees -> batched sq+exp -> prods+accum ----
        accP = psum.tile([P, 4, DL, W], f32, tag="accP")
        NFR = 2 * len(PAIRS)

        def tree(dst, hi, lo, op):
            m2 = trans.tile([P, 2, DL, W], bf16, tag="m2")
            TT(m2[:], hi, lo, op)
            TT(dst, m2[:, 0], m2[:, 1], op)

        tcount = 0

        def trees_sq_exp(batch):
            nb = len(batch)
            r2 = float(sum(v * v for v in PAIRS[batch[0]]))
            m1p = trans.tile([P, 2 * nb, DL, W], bf16, tag=f"m1p{nb}")
            for bi, pi in enumerate(batch):
                for fr in range(2):
                    mop = ALU.max if fr == 0 else ALU.min
                    kind, val = subt[(pi, fr)]
                    dst = m1p[:, 2 * bi + fr]
                    if kind == "halves":
                        tree(dst, val[0][:], val[1][:], mop)
                    else:
                        tree(dst, val[:, 0:2], val[:, 2:4], mop)
            sqp = trans.tile([P, 2 * nb, DL, W], bf16, tag=f"sqp{nb}")
            nc.scalar.activation(sqp[:], m1p[:], AF.Square)
            up = upool.tile([P, 2 * nb, DL, W], bf16, tag=f"up{nb}")
            nc.scalar.activation(up[:], sqp[:], AF.Exp,
                                 bias=bias_t[r2][:], scale=-0.5)
            return up

        def prods_accum(batch, up):
            nonlocal tcount
            for bi, pi in enumerate(batch):
                i, j, k = PAIRS[pi]
                for fr, sgn in ((0, 1), (1, -1)):
                    st, sp = (tcount == 0), (tcount == NFR - 1)
                    tcount += 1
                    si, sj, sk = sgn * i, sgn * j, sgn * k
                    mview = msk_h[sj][:, 0:3, 1 + si:1 + si + DL,
                                      1 + sk:1 + sk + W]
                    uf = up[:, 2 * bi + fr:2 * bi + fr + 1]
                    ub = uf.broadcast_to((P, 3, DL, W))
                    prods = trans.tile([P, 3, DL, W], bf16, tag="prods")
                    TT(prods[:], ub, mview, ALU.mult)
                    for ci in range(3):
                        nc.tensor.matmul(accP[:, ci], eye, prods[:, ci],
                                         start=st, stop=sp)
                    nc.tensor.matmul(accP[:, 3], eye, up[:, 2 * bi + fr],
                                     start=st, stop=sp)

        # Software-pipelined emission: PE subs lead their batch by one,
        # prods lag their batch by one, so every engine's program stays
        # stocked with ready work (no cross-engine head-of-line stalls).
        if not PESUBS_EARLY:
            pe_subs_for(BATCHES[0])
        pend = None  # (batch, up) awaiting prods
        for idx, batch in enumerate(BATCHES):
            if not PESUBS_EARLY and idx + 1 < len(BATCHES):
                pe_subs_for(BATCHES[idx + 1])
            up = trees_sq_exp(batch)
            if idx == 0:
                e2 = cpool.tile([P, 2, DL, W], bf16, tag="e2")
                TT(e2[:], expx[:, 0:2], expx[:, 2:4], ALU.add)
                esum = cpool.tile([P, DL, W], bf16, tag="esum")
                TT(esum[:], e2[:, 0], e2[:, 1], ALU.add)
            if pend is not None:
                prods_accum(*pend)
            pend = (batch, up)
        prods_accum(*pend)

        # ---- y-path (engine via env; Pool couples Pool->DVE mid-program) ----
        _ypath = nc.gpsimd if os.environ.get("V2_YPOOL", "1") == "1" \
            else nc.vector
        ym4 = cpool.tile([P, 4, DL, W], bf16, tag="ym4")
        _ypath.tensor_tensor(ym4[:], cv(masks, 0, 0), dx4[:], ALU.mult)
        v1 = cpool.tile([P, 2, DL, W], bf16, tag="v1")
        _ypath.tensor_tensor(v1[:], ym4[:, 0:2], ym4[:, 2:4], ALU.add)
        yj = cpool.tile([P, DL, W], bf16, tag="yj")
        nc.vector.scalar_tensor_tensor(yj[:], v1[:, 0], 0.0, v1[:, 1],
                                       ALU.add, ALU.add,
                                       accum_out=part[:, 2:3])

        # ---- readout + T partial ----
        TP4PSUM = os.environ.get("V2_TP4PSUM", "1") == "1"
        if not TP4PSUM:
            acc4 = cpool.tile([P, 4, DL, W], bf16, tag="acc4")
            nc.scalar.copy(acc4[:], accP[:])
        rS = cpool.tile([P, DL, W],
                        bf16 if os.environ.get("V2_RSBF", "0") == "1" else f32,
                        tag="rS")
        if RECIP_PSUM:
            nc.vector.reciprocal_approx_fast(rS[:], accP[:, 3])
        else:
            Sf = cpool.tile([P, DL, W], f32, tag="Sf")
            nc.scalar.copy(Sf[:], accP[:, 3])
            nc.vector.reciprocal_approx_fast(rS[:], Sf[:])

        lse = cpool.tile([P, DL, W], bf16, tag="lse")
        if LSE_ACCUM:
            nc.scalar.activation(lse[:], esum[:], AF.Ln,
                                 accum_out=part[:, 0:1])
        else:
            nc.scalar.activation(lse[:], esum[:], AF.Ln)
            wjl = cpool.tile([P, DL, W], bf16, tag="wjl")
            nc.vector.scalar_tensor_tensor(
                wjl[:], lse[:], 0.0, lse[:], ALU.mult, ALU.add,
                accum_out=part[:, 0:1])

        tp4 = cpool.tile([P, 4, DL, W], bf16, tag="tp4")
        TT(tp4[:], accP[:] if TP4PSUM else acc4[:], dx4[:], ALU.mult)
        u1 = cpool.tile([P, 2, DL, W], bf16, tag="u1")
        TT(u1[:], tp4[:, 0:2], tp4[:, 2:4], ALU.add)
        Tt = cpool.tile([P, DL, W], bf16, tag="Tt")
        TT(Tt[:], u1[:, 0], u1[:, 1], ALU.add)
        wj = cpool.tile([P, DL, W], bf16, tag="wj")
        if USE_TTR:
            nc.vector.tensor_tensor_reduce(wj[:], Tt[:], rS[:], -0.5, 0.0,
                                           ALU.mult, ALU.add,
                                           accum_out=part[:, 1:2])
        else:
            w1 = cpool.tile([P, DL, W], f32, tag="w1")
            TT(w1[:], Tt[:], rS[:], ALU.mult)
            nc.vector.tensor_scalar(wj[:], w1[:], -0.5, 0.0, ALU.mult,
                                    ALU.add, accum_out=part[:, 1:2])
        nc.sync.dma_start(out_d[:, :], part[:])

    nc.compile()
    return nc


def _get_nc():
    if "nc" not in _CACHED:
        _CACHED["nc"] = _build_nc()
    return _CACHED["nc"]


def make_in_maps(inputs, labels, images):
    """Host-side shard + layout prep: (b,h)->partition transpose, d/w halo
    padding, h-shifted copies (clamped), one-hot masks w/ ones channel,
    bf16 pre-cast."""
    import ml_dtypes

    bf = ml_dtypes.bfloat16
    img = np.asarray(images, np.float32)
    lab = np.asarray(labels).astype(np.int32)
    lgt = np.asarray(inputs, np.float32)

    hp = np.minimum(np.arange(H) + 1, H - 1)
    hm = np.maximum(np.arange(H) - 1, 0)

    img_p = np.pad(img, ((0, 0), (0, 0), (1, 1), (0, 0), (1, 1)),
                   mode="edge").astype(bf)                    # [B,C,D+2,H,W+2]
    lab_p = np.pad(lab, ((0, 0), (1, 1), (0, 0), (1, 1)), mode="edge")
    msk_p = np.empty((B, 4, D + 2, H, W + 2), dtype=bf)       # ones ch at [3]
    for c in (1, 2, 3):
        msk_p[:, c - 1] = (lab_p == c).astype(bf)
    msk_p[:, 3] = bf(1.0)

    def shard(arr, k, hidx=None):
        a = arr[:, :, k * DL:k * DL + DE]
        if hidx is not None:
            a = a[:, :, :, hidx]
        return np.ascontiguousarray(
            a.transpose(0, 3, 1, 2, 4)).reshape(P, -1)

    lgtb = lgt.astype(bf)
    in_maps = []
    for k in range(NCORES):
        xm = np.ascontiguousarray(
            lgtb[:, :, k * DL:(k + 1) * DL].transpose(0, 3, 1, 2, 4)
        ).reshape(P, -1)
        in_maps.append({
            "imgc": shard(img_p, k),
            "imghp": shard(img_p, k, hp),
            "imghm": shard(img_p, k, hm),
            "mskc": shard(msk_p, k),
            "mskhp": shard(msk_p[:, 0:3], k, hp),
            "mskhm": shard(msk_p[:, 0:3], k, hm),
            "logits": xm,
            "eye": _mats(),
        })
    return in_maps


def _mats():
    """[I, -I] as one [P, 2P] bf16 array."""
    import ml_dtypes

    eye = np.eye(P, dtype=np.float32)
    out = np.concatenate([eye, -eye], axis=1)
    return np.ascontiguousarray(out).astype(ml_dtypes.bfloat16)


def kernel(inputs, labels, images):
    from concourse.bass_utils import run_bass_kernel_spmd

    nc = _get_nc()
    in_maps = make_in_maps(inputs, labels, images)
    res = run_bass_kernel_spmd(nc, in_maps, core_ids=list(range(NCORES)))
    total = 0.0
    for k in range(NCORES):
        p = res.results[k]["out"].astype(np.float64)
        total += p[:, 0].sum() + p[:, 1].sum() - 0.5 * p[:, 2].sum()
    return np.float32(total / NVOX)
